# revision 2
# baseline (speedup 1.0000x reference)
"""Trainium2 Bass kernel v2 for nn_R_GAMLP_RLU (GAMLP recursive-label head).

Layout/strategy (per core; N padded 100000->100352, 12544 nodes/core):
  - 25 tiles: 24 x T=512 (SUB=128, NS=4) + 1 x T=256 (NS=2, 212 real nodes).
  - Dual feature upload (bf16): featT (feature-major, proj streams) and
    featN (node-major [t, p, s, h, 256f], weighted-sum streams).
  - Projections: stationary w2c [128,32] held across all hops, 512-col
    streams into psP [128, 3, 512] (hop h -> row 32*(h%4), view h//4).
  - Scores to node-major via 12 full [128,128] PE transposes (bf16).
  - Attention recurrence on DVE ([128, NS] ops), exp on ACT.
  - Weighted sum: stationary diag(w_h) (built by DVE/ACT/POOL tensor_scalar
    from idb), stream featN [128, 256] chunks, PSUM-accumulate right per s;
    right -> SBUF -> PE transpose -> rT (feature-major) for the MLP.
  - MLPs: per-128-out-chunk streams of 512 cols; GCNII residual adds on DVE
    (no PE identity matmuls); PReLU on ACT straight out of PSUM.
  - Output written feature-major [t, C, T] f32 straight from psO via ACT
    bias-add + DMA; host transposes to [N, C].
"""

import sys

if "/opt/trn_rl_repo" not in sys.path:
    sys.path.insert(0, "/opt/trn_rl_repo")

import numpy as np
import ml_dtypes

import bass_rust
import concourse.bass as bass
import concourse.mybir as mybir
from concourse import tile
from concourse.bass_utils import run_bass_kernel_spmd

BF16 = mybir.dt.bfloat16
F32 = mybir.dt.float32
bfnp = ml_dtypes.bfloat16
OP = mybir.AluOpType
AF = mybir.ActivationFunctionType

H, F, HID, C = 10, 256, 512, 47
N_FULL = 100000
N_CORES = 8
T0 = 512                  # full tile nodes
NT_FULL = 25              # tiles per core (24 full + 1 tail)
TAIL_T = 256              # padded tail tile size
NPC = 24 * T0 + TAIL_T    # 12544 nodes per core (padded)
LEAK = 0.2
GCN_ALPHA = 0.5

_TC = tile.TileContext

_WAIT_CAP = 1


def _split_sync_waits(nc):
    """Hoist excess per-instruction sem waits onto same-engine
    InstEventSemaphore carriers (walrus caps sync waits at 1/instruction)."""
    n = 0
    for fn in nc.m.functions:
        for bb in fn.blocks:
            insts = bb.instructions
            i = 0
            while i < len(insts):
                inst = insts[i]
                si = inst.sync_info
                waits = list(si.on_wait) if si else []
                if len(waits) > _WAIT_CAP:
                    upd = list(si.on_update) if si else []
                    extra, keep = waits[:-_WAIT_CAP], waits[-_WAIT_CAP:]
                    carriers = []
                    for k in range(0, len(extra), _WAIT_CAP):
                        nop = mybir.InstEventSemaphore(
                            name=f"wsplit_{n}", ins=[], outs=[]
                        )
                        n += 1
                        nop.engine = inst.engine
                        nop.sync_info = bass_rust.SyncInfo(
                            on_wait=extra[k : k + _WAIT_CAP], on_update=[]
                        )
                        nc.register_instruction(nop, overwrite=True)
                        carriers.append(nop)
                    inst.sync_info = bass_rust.SyncInfo(on_wait=keep, on_update=upd)
                    insts[i:i] = carriers
                    i += len(carriers)
                i += 1
    return n


def build(n_tiles, ba, a_out, a_lab):
    nc = bass.Bass()

    featT = nc.dram_tensor("featT", [n_tiles, 128, 2, H, T0], BF16, kind="ExternalInput")
    featN = nc.dram_tensor("featN", [n_tiles, 128, 4, H, F], BF16, kind="ExternalInput")
    embT = nc.dram_tensor("embT", [n_tiles, C + 1, T0], BF16, kind="ExternalInput")
    w2d = nc.dram_tensor("w2", [128, 2, 32], BF16, kind="ExternalInput")
    w0d = nc.dram_tensor("w0", [128, 2, HID], BF16, kind="ExternalInput")
    wg1d = nc.dram_tensor("wg1", [128, 4, HID], BF16, kind="ExternalInput")
    wg2d = nc.dram_tensor("wg2", [128, 4, HID], BF16, kind="ExternalInput")
    wlastd = nc.dram_tensor("wlast", [128, 4, C], BF16, kind="ExternalInput")
    wl0d = nc.dram_tensor("wl0", [C + 1, HID], BF16, kind="ExternalInput")
    wl1d = nc.dram_tensor("wl1", [128, 4, HID], BF16, kind="ExternalInput")
    wl2d = nc.dram_tensor("wl2", [128, 4, HID], BF16, kind="ExternalInput")
    wl3d = nc.dram_tensor("wl3", [128, 4, C], BF16, kind="ExternalInput")
    b0d = nc.dram_tensor("b0t", [128, 4], F32, kind="ExternalInput")
    b0hd = nc.dram_tensor("b0h", [128, 4], F32, kind="ExternalInput")
    bl0d = nc.dram_tensor("bl0t", [128, 4], F32, kind="ExternalInput")
    bl1d = nc.dram_tensor("bl1t", [128, 4], F32, kind="ExternalInput")
    bl2d = nc.dram_tensor("bl2t", [128, 4], F32, kind="ExternalInput")
    bfind = nc.dram_tensor("bfin", [C, 1], F32, kind="ExternalInput")
    idbd = nc.dram_tensor("idb", [128, 128], BF16, kind="ExternalInput")

    outd = nc.dram_tensor("outT", [n_tiles, C, T0], F32, kind="ExternalOutput")

    with _TC(nc) as tc:
        with (
            tc.tile_pool(name="consts", bufs=1) as cp,
            tc.tile_pool(name="feat", bufs=2) as fp,
            tc.tile_pool(name="act", bufs=1) as ap,
            tc.tile_pool(name="small", bufs=2) as sp,
            tc.tile_pool(name="ps", bufs=1, space="PSUM") as ps,
        ):
            # ---- constants ----
            idb = cp.tile([128, 128], BF16)
            w2 = cp.tile([128, 2, 32], BF16)
            w0 = cp.tile([128, 2, HID], BF16)
            wg1 = cp.tile([128, 4, HID], BF16)
            wg2 = cp.tile([128, 4, HID], BF16)
            wlast = cp.tile([128, 4, C], BF16)
            wl0 = cp.tile([C + 1, HID], BF16)
            wl1 = cp.tile([128, 4, HID], BF16)
            wl2 = cp.tile([128, 4, HID], BF16)
            wl3 = cp.tile([128, 4, C], BF16)
            b0 = cp.tile([128, 4], F32)
            b0h = cp.tile([128, 4], F32)
            bl0 = cp.tile([128, 4], F32)
            bl1 = cp.tile([128, 4], F32)
            bl2 = cp.tile([128, 4], F32)
            bfin = cp.tile([C, 1], F32)

            nc.sync.dma_start(idb[:], idbd[:])
            nc.sync.dma_start(w2[:], w2d[:])
            nc.sync.dma_start(w0[:], w0d[:])
            nc.sync.dma_start(wg1[:], wg1d[:])
            nc.sync.dma_start(wg2[:], wg2d[:])
            nc.sync.dma_start(wlast[:], wlastd[:])
            nc.sync.dma_start(wl0[:], wl0d[:])
            nc.sync.dma_start(wl1[:], wl1d[:])
            nc.sync.dma_start(wl2[:], wl2d[:])
            nc.sync.dma_start(wl3[:], wl3d[:])
            nc.sync.dma_start(b0[:], b0d[:])
            nc.sync.dma_start(b0h[:], b0hd[:])
            nc.sync.dma_start(bl0[:], bl0d[:])
            nc.sync.dma_start(bl1[:], bl1d[:])
            nc.sync.dma_start(bl2[:], bl2d[:])
            nc.sync.dma_start(bfin[:], bfind[:])

            def front_a(t, T, NS):
                """loads + projections + score transposes + xn."""
                fT = fp.tile([128, 2, H, T0], BF16, tag="fT", name=f"fT_{t}")
                fN = fp.tile([128, 4, H, F], BF16, tag="fN", bufs=3, name=f"fN_{t}")
                emb = fp.tile([C + 1, T0], BF16, tag="emb", bufs=3, name=f"emb_{t}")
                for k in range(5):
                    # full-T0 loads so the APs merge to <=3 dims (tail tile
                    # loads its zero padding too; harmless)
                    nc.sync.dma_start(
                        fT[:, :, 2 * k : 2 * k + 2, :],
                        featT[t, :, :, 2 * k : 2 * k + 2, :],
                    )
                    nc.gpsimd.dma_start(
                        fN[:, :NS, 2 * k : 2 * k + 2],
                        featN[t, :, :NS, 2 * k : 2 * k + 2],
                    )
                nc.scalar.dma_start(emb[:, :T], embT[t, :, :T])

                # projections per view v (hops 4v..4v+3); psP ring of
                # [128, T0] banks; transposes for view v emitted after view
                # v+1's matmuls so the PSUM->SBUF copy is off the PE critical
                # path. xn[n, v, s, j, c]: hop h=4v+j, c=0:xl, 1:xr.
                xn = sp.tile([128, 3, 4, 4, 2], BF16, tag="xn", name=f"xn_{t}")
                psT = ps.tile([128, 4, 4, 16, 2], BF16, tag="psT",
                              name=f"psT_{t}")
                views = []
                for v in range(3):
                    hops = list(range(4 * v, min(4 * v + 4, H)))
                    psPv = ps.tile([128, T0], F32, tag="psP", bufs=2,
                                   name=f"psP_{t}_{v}")
                    for c in range(2):
                        for h in hops:
                            r = 32 * (h % 4)
                            nc.tensor.matmul(
                                psPv[r : r + 32, :T],
                                w2[:, c, :],
                                fT[:, c, h, :T],
                                start=(c == 0),
                                stop=(c == 1),
                                tile_position=(0, r),
                            )
                    xlxr = ap.tile([128, T0], BF16, tag="xlxr", bufs=3,
                                   name=f"xlxr_{t}_{v}")
                    nr = 32 * len(hops)
                    nc.vector.tensor_copy(xlxr[:nr, :T], psPv[:nr, :T])
                    views.append((v, hops, nr, xlxr))
                for v, hops, nr, xlxr in views:
                    nj = len(hops)
                    for si in range(NS):
                        nc.tensor.transpose(
                            psT[:, si, :nj],
                            xlxr[:nr, si * 128 : (si + 1) * 128],
                            idb[:nr, :nr],
                        )
                    nc.vector.tensor_copy(
                        xn[:, v, :NS, :nj], psT[:, :NS, :nj, 0, :]
                    )
                return t, T, NS, fN, emb, xn

            def rec_gen(st, out_box):
                """attention recurrence on POOL (+ACT exps, one DVE recip per
                hop), then wb + diag tiles; emitted as a generator so its ops
                interleave into the back/wsum streams (avoids head-of-line
                blocking on any one queue)."""
                t, T, NS, fN, emb, xn = st

                def xl(i):
                    return xn[:, i // 4, :NS, i % 4, 0]

                def xr(i):
                    return xn[:, i // 4, :NS, i % 4, 1]

                sc = sp.tile([128, 4, H], F32, tag="sc", name=f"sc_{t}")
                ex = sp.tile([128, 4, H], F32, tag="ex", name=f"ex_{t}")
                num = sp.tile([128, 4], F32, tag="num", name=f"num_{t}")
                den = sp.tile([128, 4], F32, tag="den", name=f"den_{t}")
                tmp = sp.tile([128, 4], F32, tag="tmp", name=f"tmp_{t}")
                tmp2 = sp.tile([128, 4], F32, tag="tmp2", name=f"tmp2_{t}")
                z = sp.tile([128, 4], F32, tag="z", name=f"z_{t}")
                rec = sp.tile([128, 4], F32, tag="rec", name=f"rec_{t}")

                nc.vector.scalar_tensor_tensor(
                    z[:, :NS], xl(0), float(ba), xr(0), op0=OP.add, op1=OP.add
                )
                nc.vector.scalar_tensor_tensor(
                    sc[:, :NS, 0], z[:, :NS], LEAK, z[:, :NS],
                    op0=OP.mult, op1=OP.max,
                )
                nc.scalar.activation(ex[:, :NS, 0], sc[:, :NS, 0], AF.Exp)
                nc.vector.tensor_copy(den[:, :NS], ex[:, :NS, 0])
                nc.vector.tensor_mul(num[:, :NS], ex[:, :NS, 0], xl(0))
                yield
                for i in range(1, H):
                    nc.vector.reciprocal(rec[:, :NS], den[:, :NS])
                    nc.vector.tensor_mul(tmp[:, :NS], num[:, :NS], rec[:, :NS])
                    nc.vector.scalar_tensor_tensor(
                        z[:, :NS], tmp[:, :NS], float(ba), xr(i),
                        op0=OP.add, op1=OP.add,
                    )
                    nc.vector.scalar_tensor_tensor(
                        sc[:, :NS, i], z[:, :NS], LEAK, z[:, :NS],
                        op0=OP.mult, op1=OP.max,
                    )
                    nc.scalar.activation(ex[:, :NS, i], sc[:, :NS, i], AF.Exp)
                    nc.vector.tensor_add(den[:, :NS], den[:, :NS],
                                         ex[:, :NS, i])
                    if i < H - 1:
                        nc.vector.tensor_mul(tmp2[:, :NS], ex[:, :NS, i],
                                             xl(i))
                        nc.vector.tensor_add(num[:, :NS], num[:, :NS],
                                             tmp2[:, :NS])
                    yield
                recf = sp.tile([128, 4], F32, tag="recf", name=f"recf_{t}")
                nc.vector.reciprocal(recf[:, :NS], den[:, :NS])
                wb = sp.tile([128, 4, H], F32, tag="wb", name=f"wb_{t}")
                for si in range(NS):
                    nc.vector.tensor_scalar_mul(
                        wb[:, si, :], ex[:, si, :], recf[:, si : si + 1]
                    )
                yield
                dgs = {}
                for si in range(NS):
                    for h in range(H):
                        k = si * H + h
                        dg = ap.tile([128, 128], BF16, tag="diag", bufs=82,
                                     name=f"dg_{t}_{si}_{h}")
                        if (k % 5) in (0, 2, 3):
                            nc.vector.tensor_scalar_mul(
                                dg[:, :], idb[:, :], wb[:, si, h : h + 1]
                            )
                        else:
                            nc.scalar.activation(
                                dg[:, :], idb[:, :], AF.Copy,
                                scale=wb[:, si, h : h + 1],
                            )
                        dgs[(si, h)] = dg
                        if k % 10 == 9:
                            yield
                out_box.append((t, T, NS, fN, emb, dgs))

            def wsum_gen(st, out_box):
                """weighted sum -> rT (feature-major): stationary fN chunks
                stream prebuilt diag(w); accumulates rT in PSUM."""
                t, T, NS, fN, emb, dgs = st
                psRT = ps.tile([128, 2, 4, 128], F32, tag="psRT",
                               name=f"psRT_{t}")
                for si in range(NS):
                    for h in range(H):
                        dg = dgs[(si, h)]
                        for c in range(2):
                            nc.tensor.matmul(
                                psRT[:, c, si, :],
                                fN[:, si, h, c * 128 : (c + 1) * 128],
                                dg[:, :],
                                start=(h == 0),
                                stop=(h == H - 1),
                                skip_group_check=True,
                            )
                    yield
                rT = ap.tile([128, 2, T0], BF16, tag="rT", bufs=2,
                             name=f"rT_{t}")
                nc.vector.tensor_copy(rT[:, :, :T], psRT[:, :, :NS])
                out_box.append((t, T, NS, emb, rT))

            def back_x_gen(st, out_box):
                """lr_output MLP (x path) through wlast -> pbF[0:C]."""
                t, T, NS, emb, rT = st
                pbF = ps.tile([128, T0], F32, tag="pbF", name=f"pbF_{t}")
                h0q = ap.tile([128, 4, T0], BF16, tag="h0q", name=f"h0q_{t}")
                xi1 = ap.tile([128, 4, T0], BF16, tag="xi", bufs=2,
                              name=f"xi1_{t}")
                for mc in range(4):
                    pb = ps.tile([128, T0], F32, tag="pb", bufs=2,
                                 name=f"pb0_{t}_{mc}")
                    for c in range(2):
                        nc.tensor.matmul(
                            pb[:, :T], w0[:, c, mc * 128 : (mc + 1) * 128],
                            rT[:, c, :T], start=(c == 0), stop=(c == 1),
                        )
                    nc.scalar.activation(
                        h0q[:, mc, :T], pb[:, :T], AF.Identity,
                        bias=b0h[:, mc : mc + 1], scale=GCN_ALPHA,
                    )
                    nc.scalar.activation(
                        xi1[:, mc, :T], pb[:, :T], AF.Prelu,
                        bias=b0[:, mc : mc + 1], alpha=float(a_out),
                    )
                    yield

                xi_in = xi1
                for gi, wg in enumerate((wg1, wg2)):
                    sup = ap.tile([128, 4, T0], BF16, tag="sup", bufs=2,
                                  name=f"sup_{t}_{gi}")
                    for mc in (0, 2):
                        nc.vector.scalar_tensor_tensor(
                            sup[:, mc : mc + 2, :T], xi_in[:, mc : mc + 2, :T],
                            1.0 - GCN_ALPHA, h0q[:, mc : mc + 2, :T],
                            op0=OP.mult, op1=OP.add,
                        )
                    xi_out = ap.tile([128, 4, T0], BF16, tag="xi", bufs=2,
                                     name=f"xi_{t}_{gi}")
                    xs4 = ap.tile([128, 4, T0], BF16, tag="xs",
                                  name=f"xs_{t}_{gi}")
                    for mc in range(4):
                        pb = ps.tile([128, T0], F32, tag="pb", bufs=2,
                                     name=f"pg_{t}_{gi}_{mc}")
                        for c in range(4):
                            nc.tensor.matmul(
                                pb[:, :T], wg[:, c, mc * 128 : (mc + 1) * 128],
                                sup[:, c, :T], start=(c == 0), stop=(c == 3),
                            )
                        nc.vector.tensor_add(
                            xs4[:, mc, :T], pb[:, :T], xi_in[:, mc, :T]
                        )
                        if mc % 2 == 1:
                            nc.scalar.activation(
                                xi_out[:, mc - 1 : mc + 1, :T],
                                xs4[:, mc - 1 : mc + 1, :T], AF.Prelu,
                                alpha=float(a_out),
                            )
                        yield
                    xi_in = xi_out

                for c in range(4):
                    nc.tensor.matmul(
                        pbF[:C, :T], wlast[:, c, :], xi_in[:, c, :T],
                        start=(c == 0), stop=False, skip_group_check=True,
                    )
                out_box.append((t, T, NS, emb, pbF))

            def back_y_gen(st_box):
                """label_fc (y path) + final combine + store."""
                t, T, NS, emb, pbF = st_box
                y_in = None
                for li, (wl, blv) in enumerate(((wl0, bl0), (wl1, bl1),
                                                (wl2, bl2))):
                    y_out = ap.tile([128, 4, T0], BF16, tag="y", bufs=2,
                                    name=f"y_{t}_{li}")
                    for mc in range(4):
                        pb = ps.tile([128, T0], F32, tag="pb", bufs=2,
                                     name=f"py_{t}_{li}_{mc}")
                        if li == 0:
                            nc.tensor.matmul(
                                pb[:, :T], wl0[:, mc * 128 : (mc + 1) * 128],
                                emb[:, :T], start=True, stop=True,
                            )
                            nc.scalar.activation(
                                y_out[:, mc, :T], pb[:, :T], AF.Prelu,
                                alpha=float(a_lab),
                            )
                        else:
                            for c in range(4):
                                nc.tensor.matmul(
                                    pb[:, :T],
                                    wl[:, c, mc * 128 : (mc + 1) * 128],
                                    y_in[:, c, :T], start=(c == 0),
                                    stop=(c == 3),
                                )
                            nc.scalar.activation(
                                y_out[:, mc, :T], pb[:, :T], AF.Prelu,
                                bias=blv[:, mc : mc + 1], alpha=float(a_lab),
                            )
                        yield
                    y_in = y_out

                for c in range(4):
                    nc.tensor.matmul(
                        pbF[:C, :T], wl3[:, c, :], y_in[:, c, :T],
                        start=False, stop=(c == 3), skip_group_check=True,
                    )
                outx = ap.tile([C, T0], F32, tag="outx", bufs=2,
                               name=f"outx_{t}")
                nc.scalar.activation(
                    outx[:, :T], pbF[:C, :T], AF.Identity, bias=bfin[:, 0:1]
                )
                nc.sync.dma_start(outd[t, :, :T], outx[:, :T])

            def drive(gens):
                gens = [g for g in gens if g is not None]
                while gens:
                    nxt = []
                    for g in gens:
                        try:
                            next(g)
                            nxt.append(g)
                        except StopIteration:
                            pass
                    gens = nxt

            def tile_dims(t):
                return (T0, 4) if t < n_tiles - 1 else (TAIL_T, 2)

            # Pipelined emission. Iteration t emits, round-robin interleaved:
            #   front_a(t+1) [PE proj/transp], back_x(t-1), wsum(t),
            #   back_y(t-1), rec(t+1) [POOL/ACT/DVE]
            rec_box, wsum_box, bx_box = [], [], []
            sa = front_a(0, *tile_dims(0))
            drive([rec_gen(sa, rec_box)])
            pend_diag = rec_box.pop()
            prev_w = None
            for t in range(n_tiles):
                if t + 1 < n_tiles:
                    sa2 = front_a(t + 1, *tile_dims(t + 1))
                    gr = rec_gen(sa2, rec_box)
                else:
                    gr = None
                gens = [wsum_gen(pend_diag, wsum_box), gr]
                if prev_w is not None:
                    gens = [back_x_gen(prev_w, bx_box), gens[0],
                            gens[1]]
                drive([g for g in gens if g is not None])
                if prev_w is not None:
                    drive([back_y_gen(bx_box.pop())])
                prev_w = wsum_box.pop()
                pend_diag = rec_box.pop() if rec_box else None
            drive([back_x_gen(prev_w, bx_box)])
            drive([back_y_gen(bx_box.pop())])

    _split_sync_waits(nc)
    return nc


def _prep_weights(inputs):
    f32 = np.float32
    wa = np.asarray(inputs["wa"], f32)
    wl, wr = wa[:F], wa[F:]
    w2 = np.zeros((128, 2, 32), bfnp)
    w2s = np.stack([wl, wr], axis=1).astype(bfnp)  # [256, 2]
    w2[:, 0, :2] = w2s[:128]
    w2[:, 1, :2] = w2s[128:]
    b0 = np.asarray(inputs["b0"], f32)
    blast = np.asarray(inputs["b_last"], f32) + np.asarray(inputs["bl3"], f32)
    r = lambda w: np.ascontiguousarray(
        np.asarray(w).astype(bfnp).reshape(-1, 128, w.shape[-1]).transpose(1, 0, 2)
    )
    m = {
        "w2": w2,
        "w0": r(inputs["w0"]),
        "wg1": r(inputs["wg1"]),
        "wg2": r(inputs["wg2"]),
        "wlast": r(inputs["w_last"]),
        "wl0": np.concatenate(
            [np.asarray(inputs["wl0"]), np.asarray(inputs["bl0"])[None, :]], 0
        ).astype(bfnp),
        "wl1": r(inputs["wl1"]),
        "wl2": r(inputs["wl2"]),
        "wl3": r(inputs["wl3"]),
        "b0t": np.ascontiguousarray(b0.reshape(4, 128).T),
        "b0h": np.ascontiguousarray((GCN_ALPHA * b0).reshape(4, 128).T),
        "bl0t": np.ascontiguousarray(np.asarray(inputs["bl0"], f32).reshape(4, 128).T),
        "bl1t": np.ascontiguousarray(np.asarray(inputs["bl1"], f32).reshape(4, 128).T),
        "bl2t": np.ascontiguousarray(np.asarray(inputs["bl2"], f32).reshape(4, 128).T),
        "bfin": np.ascontiguousarray(blast.reshape(C, 1)),
        "idb": np.eye(128, dtype=bfnp),
    }
    return m


def _shard_maps(inputs, n_tiles, n_cores):
    feats = np.asarray(inputs["features"])
    lab = np.asarray(inputs["label_emb"])
    n_pad = n_cores * NPC
    fb = np.zeros((H, n_pad, F), bfnp)
    fb[:, :N_FULL] = feats.astype(bfnp)
    lb = np.zeros((n_pad, C), bfnp)
    lb[:N_FULL] = lab.astype(bfnp)
    wmap = _prep_weights(inputs)
    maps = []
    for core in range(n_cores):
        sl = slice(core * NPC, (core + 1) * NPC)
        fsh = fb[:, sl, :]  # [H, NPC, F]
        lsh = lb[sl]
        # pad node dim to n_tiles*T0 for uniform tile arrays
        fpad = np.zeros((H, n_tiles * T0, F), bfnp)
        fpad[:, :NPC] = fsh
        lpad = np.zeros((n_tiles * T0, C + 1), bfnp)
        lpad[:NPC, :C] = lsh
        lpad[:, C] = 1.0
        # featT[t, p, c, h, n]: feature f = c*128+p
        fT = np.ascontiguousarray(
            fpad.reshape(H, n_tiles, T0, 2, 128).transpose(1, 4, 3, 0, 2)
        )
        # featN[t, p, s, h, f]: node n = t*T0 + s*128 + p
        fN = np.ascontiguousarray(
            fpad.reshape(H, n_tiles, 4, 128, F).transpose(1, 3, 2, 0, 4)
        )
        eT = np.ascontiguousarray(
            lpad.reshape(n_tiles, T0, C + 1).transpose(0, 2, 1)
        )
        m = dict(wmap)
        m["featT"] = fT
        m["featN"] = fN
        m["embT"] = eT
        maps.append(m)
    return maps


_CACHE = {}


def _get_nc(n_tiles, ba, a_out, a_lab):
    key = (n_tiles, round(float(ba), 8), round(float(a_out), 8),
           round(float(a_lab), 8))
    if key not in _CACHE:
        _CACHE[key] = build(n_tiles, float(ba), float(a_out), float(a_lab))
    return _CACHE[key]


def kernel(**inputs) -> np.ndarray:
    ba = float(np.asarray(inputs["ba"]))
    a_out = float(np.asarray(inputs["a_out"]))
    a_lab = float(np.asarray(inputs["a_lab"]))
    nc = _get_nc(NT_FULL, ba, a_out, a_lab)
    maps = _shard_maps(inputs, NT_FULL, N_CORES)
    res = run_bass_kernel_spmd(nc, maps, list(range(N_CORES)))
    outs = []
    for i in range(N_CORES):
        o = np.asarray(res.results[i]["outT"], np.float32)  # [nt, C, T0]
        o = o.transpose(0, 2, 1).reshape(-1, C)  # [nt*T0, C]
        outs.append(o[:NPC])
    full = np.concatenate(outs, axis=0)
    return full[:N_FULL]
